# revision 45
# baseline (speedup 1.0000x reference)
"""Trainium2 Bass kernel for AdaptiveAttention.

out = softmax((Q @ K^T) * scale * sigmoid(span)) @ V
B=4, Sq=Sk=D=2048, fp32 I/O, bf16 TensorE compute.

Sharding: (batch, query-half) across 8 NeuronCores — each core owns a
[1024, 2048] slice of the output and needs no cross-core communication
(softmax reduces over keys, which are fully local).

Per-core algorithm:
  - gate[k] = sigmoid(span[k]) / sqrt(D) is folded into K rows during the
    fp32->bf16 convert (per-partition scalar multiply).
  - Q/K/V are loaded with GpSimd SWDGE cast-DMAs (fp32 DRAM -> bf16
    SBUF, no compute-engine converts); Q^T / K^T are produced with PE
    transposes (batched 8 blocks per PSUM bank, copied out on ACT); P^T
    uses the DMA XBAR transpose in phase 2 where the DMA fabric is
    otherwise idle.  Keeping the XBAR off the phase-1 critical path
    avoids xbar-mode serialization against the input load stream.
  - softmax skips the max-subtraction (scores are ~N(0, 0.73); exp is
    safe in fp32) and defers normalization to a per-row reciprocal
    multiply on the PV output.
"""

import sys

import numpy as np

if "/opt/trn_rl_repo" not in sys.path:
    sys.path.insert(0, "/opt/trn_rl_repo")

B = 4
SEQ = 2048
D = 2048
N_CORES = 8
Q_SHARD = (B * SEQ) // N_CORES  # 1024 query rows per core

_CACHE: dict = {}


def build(q_rows: int = Q_SHARD, seq: int = SEQ, d: int = D):
    """Build + compile the per-core Bass graph (same graph on all cores)."""
    import ml_dtypes

    import concourse.tile as tile
    from concourse import bacc, mybir

    f32 = mybir.dt.float32
    bf16 = mybir.dt.bfloat16
    AF = mybir.ActivationFunctionType

    P = 128
    n_qt = q_rows // P
    n_kt = seq // P
    n_dt = d // P
    KC = 512  # k-chunk width (one PSUM bank of fp32 scores)
    n_kc = seq // KC
    kt_per_kc = KC // P
    DC = 512  # output d-chunk width
    n_dc = d // DC
    TB = 8  # transpose blocks batched per PSUM bank
    scale = 1.0 / float(np.sqrt(d))

    nc = bacc.Bacc("TRN2", target_bir_lowering=False, debug=False)
    q_d = nc.dram_tensor("q", [q_rows, d], f32, kind="ExternalInput").ap()
    k_d = nc.dram_tensor("k", [seq, d], f32, kind="ExternalInput").ap()
    v_d = nc.dram_tensor("v", [seq, d], f32, kind="ExternalInput").ap()
    span_d = nc.dram_tensor("span", [1, seq], f32, kind="ExternalInput").ap()
    out_d = nc.dram_tensor("out", [q_rows, d], f32, kind="ExternalOutput").ap()

    with tile.TileContext(nc) as tc:
        with tc.tile_pool(name="singles", bufs=1) as singles, \
             tc.tile_pool(name="cv", bufs=13) as cvp, \
             tc.tile_pool(name="ktp", bufs=2) as ktp, \
             tc.tile_pool(name="ptp", bufs=2) as ptp, \
             tc.tile_pool(name="obp", bufs=2) as obp, \
             tc.tile_pool(name="trps", bufs=2, space="PSUM") as trps:

            # Identity as a NEFF-embedded constant: loads on the HWDGE
            # queue so the GpSimd SWDGE (cast-DMA) queue starts immediately.
            ident_dram = nc.inline_tensor(
                np.eye(P, dtype=ml_dtypes.bfloat16), name="ident_c"
            )
            ident = singles.tile([P, P], bf16, tag="ident")
            nc.sync.dma_start(out=ident, in_=ident_dram.ap())

            # gate[p, t] = sigmoid(span[t*128 + p]) * scale.
            # Load the span row contiguously (one descriptor -- the
            # [1,2048]->[128,16] gather costs ~25us of 4-byte descriptors
            # and was the prologue critical path), sigmoid on the row,
            # then distribute to partitions with 16 tiny PE outer-product
            # matmuls against a [1,1] ones tile.
            span_row = singles.tile([1, seq], f32, tag="span_row")
            nc.sync.dma_start(out=span_row, in_=span_d)
            nc.scalar.activation(out=span_row, in_=span_row, func=AF.Sigmoid)
            sig_row = span_row
            ones11 = singles.tile([1, 1], f32, tag="ones11")
            nc.vector.memset(ones11, 1.0)
            gate = singles.tile([P, n_kt], f32, tag="gate")

            # Persistent bf16 tensors
            QT = singles.tile([P, n_dt, q_rows], bf16, tag="QT")  # [d, dt, q]
            Vb = singles.tile([P, n_kt, d], bf16, tag="Vb")       # [k, kt, d]
            Pm = singles.tile([P, n_qt, seq], bf16, tag="Pm")     # [q, qt, k]
            sums = singles.tile([P, n_qt, n_kc], f32, tag="sums")

            # Loads are split into transpose-group-sized pieces so each
            # PE transpose group can start as soon as its piece lands.
            tb = min(TB, n_dt)
            n_tg = n_dt // tb
            GW = tb * P  # columns per transpose group

            def load_q(qt):
                # SWDGE cast-DMA: fp32 DRAM -> bf16 SBUF directly
                pieces = []
                for g in range(n_tg):
                    t = cvp.tile([P, GW], bf16, tag="cv", name=f"qr{qt}_{g}")
                    nc.gpsimd.dma_start(
                        out=t, in_=q_d[qt * P:(qt + 1) * P, g * GW:(g + 1) * GW]
                    )
                    pieces.append(t)
                return pieces

            def load_k(kt):
                pieces = []
                for g in range(n_tg):
                    t = cvp.tile([P, GW], bf16, tag="cv", name=f"kr{kt}_{g}")
                    nc.gpsimd.dma_start(
                        out=t, in_=k_d[kt * P:(kt + 1) * P, g * GW:(g + 1) * GW]
                    )
                    # per-key gate fold in place (bf16 4x DVE mode)
                    nc.vector.tensor_scalar_mul(t, t, gate[:, kt:kt + 1])
                    pieces.append(t)
                return pieces

            def load_v(kt):
                nc.gpsimd.dma_start(out=Vb[:, kt, :], in_=v_d[kt * P:(kt + 1) * P, :])

            def pe_transpose(pieces, dst, dst_col, copy_engine):
                """dst[:, dt, dst_col*128 : +128] = pieces^T blocks.

                pieces: n_tg tiles of [128, tb*128] bf16 (natural layout).
                Batches tb 128x128 PE transposes per PSUM bank, then one
                copy per bank to SBUF.
                """
                for g in range(n_tg):
                    tr = trps.tile([P, tb, P], bf16, tag="tr")
                    for j in range(tb):
                        nc.tensor.transpose(
                            tr[:, j, :], pieces[g][:, j * P:(j + 1) * P], ident
                        )
                    copy_engine(
                        out=dst[:, g * tb:(g + 1) * tb,
                                dst_col * P:(dst_col + 1) * P],
                        in_=tr,
                    )

            def s_block(kc, qt, KT):
                s_ps = spsum.tile([P, KC], f32, tag="s")
                for dt in range(n_dt):
                    nc.tensor.matmul(
                        s_ps,
                        QT[:, dt, qt * P:(qt + 1) * P],
                        KT[:, dt, :],
                        start=(dt == 0),
                        stop=(dt == n_dt - 1),
                    )
                nc.scalar.activation(
                    out=Pm[:, qt, kc * KC:(kc + 1) * KC],
                    in_=s_ps,
                    func=AF.Exp,
                    accum_out=sums[:, qt, kc:kc + 1],
                )

            gp = tc.tile_pool(name="gatep", bufs=1, space="PSUM")
            gpp = gp.__enter__()
            gate_ps = gpp.tile([P, n_kt], f32, tag="gps")
            for t in range(n_kt):
                nc.tensor.matmul(
                    gate_ps[:, t:t + 1],
                    sig_row[0:1, t * P:(t + 1) * P],
                    ones11,
                    start=True,
                    stop=True,
                )
            nc.scalar.mul(out=gate, in_=gate_ps, mul=scale)
            gp.__exit__(None, None, None)

            ph1 = tc.tile_pool(name="spsum", bufs=5, space="PSUM")
            spsum = ph1.__enter__()
            # ---- Phase 1: S = gated Q K^T, P = exp(S) ------------------
            # SWDGE (cast-DMA) queue is FIFO: emit loads in consumption
            # order (K chunk 0 first -- its 64 PE transposes are the long
            # pole to the first S matmul), prefetch K one chunk ahead, and
            # push all V loads to the tail (phase 2 consumes V much later).
            # The next chunk's transposes are interleaved into the current
            # chunk's S loop so KT production never stalls the PE.
            kbs = {0: [load_k(j) for j in range(kt_per_kc)]}
            qb0 = load_q(0)
            qb1 = load_q(1)
            if n_kc > 1:
                kbs[1] = [load_k(kt_per_kc + j) for j in range(kt_per_kc)]

            KTs = {}
            ktcopy = nc.scalar.copy

            def mk_tk(kc, j):
                if j == 0:
                    KTs[kc] = ktp.tile([P, n_dt, KC], bf16, tag="KT", name=f"KT{kc}")
                pe_transpose(kbs[kc][j], KTs[kc], j, ktcopy)

            for j in range(kt_per_kc):
                mk_tk(0, j)
            pe_transpose(qb0, QT, 0, nc.scalar.copy)
            pe_transpose(qb1, QT, 1, nc.scalar.copy)
            if n_qt > 2:  # transpose Q several tiles ahead of its S block
                pe_transpose(load_q(2), QT, 2, nc.scalar.copy)
            for qt in range(n_qt):
                s_block(0, qt, KTs[0])
                if qt + 3 < n_qt:
                    pe_transpose(load_q(qt + 3), QT, qt + 3, nc.scalar.copy)
                if qt - (n_qt - kt_per_kc) >= 0 and n_kc > 1:
                    mk_tk(1, qt - (n_qt - kt_per_kc))
                if qt == n_qt - 1 and n_kc > 2:
                    kbs[2] = [
                        load_k(2 * kt_per_kc + j) for j in range(kt_per_kc)
                    ]

            for kc in range(1, n_kc):
                for qt in range(n_qt):
                    s_block(kc, qt, KTs[kc])
                    if qt < kt_per_kc and kc + 1 < n_kc:
                        mk_tk(kc + 1, qt)
                    if qt == 0 and kc + 2 < n_kc:
                        kbs[kc + 2] = [
                            load_k((kc + 2) * kt_per_kc + j)
                            for j in range(kt_per_kc)
                        ]

            for vt in range(n_kt):
                load_v(vt)

            ph1.__exit__(None, None, None)
            ph2 = tc.tile_pool(name="opsum", bufs=6, space="PSUM")
            opsum = ph2.__enter__()

            rowsum = singles.tile([P, n_qt], f32, tag="rowsum")
            nc.vector.tensor_reduce(
                out=rowsum, in_=sums, axis=mybir.AxisListType.X,
                op=mybir.AluOpType.add,
            )
            rinv = singles.tile([P, n_qt], f32, tag="rinv")
            nc.vector.reciprocal(rinv, rowsum)

            # ---- Phase 2: O[qt] = (P[qt] @ V) * rinv[qt] ---------------
            for qt in range(n_qt):
                PT = ptp.tile([P, n_kt, P], bf16, tag="PT")
                nc.scalar.dma_start_transpose(out=PT, in_=Pm[:, qt, :])
                for dc in range(n_dc):
                    o_ps = opsum.tile([P, DC], f32, tag="o")
                    for kt in range(n_kt):
                        nc.tensor.matmul(
                            o_ps,
                            PT[:, kt, :],
                            Vb[:, kt, dc * DC:(dc + 1) * DC],
                            start=(kt == 0),
                            stop=(kt == n_kt - 1),
                        )
                    ob = obp.tile([P, DC], f32, tag="ob")
                    nc.vector.tensor_scalar_mul(ob, o_ps, rinv[:, qt:qt + 1])
                    nc.sync.dma_start(
                        out=out_d[qt * P:(qt + 1) * P, dc * DC:(dc + 1) * DC],
                        in_=ob,
                    )

            ph2.__exit__(None, None, None)

    nc.compile()
    return nc


def _get_compiled():
    if "nc" not in _CACHE:
        _CACHE["nc"] = build()
    return _CACHE["nc"]


def _shard_inputs(query, key, value, span):
    in_maps = []
    for c in range(N_CORES):
        b, h = c // 2, c % 2
        in_maps.append({
            "q": np.ascontiguousarray(
                query[b, h * Q_SHARD:(h + 1) * Q_SHARD], dtype=np.float32
            ),
            "k": np.ascontiguousarray(key[b], dtype=np.float32),
            "v": np.ascontiguousarray(value[b], dtype=np.float32),
            "span": np.ascontiguousarray(span, dtype=np.float32),
        })
    return in_maps


def kernel(**inputs) -> np.ndarray:
    query = np.asarray(inputs["query"], dtype=np.float32)
    key = np.asarray(inputs["key"], dtype=np.float32)
    value = np.asarray(inputs["value"], dtype=np.float32)
    span = np.asarray(inputs["span_param"], dtype=np.float32)

    from concourse.bass_utils import run_bass_kernel_spmd

    nc = _get_compiled()
    in_maps = _shard_inputs(query, key, value, span)
    res = run_bass_kernel_spmd(nc, in_maps, core_ids=list(range(N_CORES)))

    out = np.empty((B, SEQ, D), dtype=np.float32)
    for c in range(N_CORES):
        b, h = c // 2, c % 2
        out[b, h * Q_SHARD:(h + 1) * Q_SHARD] = res.results[c]["out"]
    return out


if __name__ == "__main__":
    rng = np.random.default_rng(0)
    inputs = {
        "query": rng.standard_normal((B, SEQ, D), dtype=np.float32),
        "key": rng.standard_normal((B, SEQ, D), dtype=np.float32),
        "value": rng.standard_normal((B, SEQ, D), dtype=np.float32),
        "span_param": np.ones((1, SEQ), dtype=np.float32),
    }
    out = kernel(**inputs)
    print(out.shape, out.dtype, float(np.abs(out).mean()))


# revision 46
# speedup vs baseline: 1.2012x; 1.2012x over previous
"""Trainium2 Bass kernel for AdaptiveAttention.

out = softmax((Q @ K^T) * scale * sigmoid(span)) @ V
B=4, Sq=Sk=D=2048, fp32 I/O, bf16 TensorE compute.

Sharding: (batch, query-half) across 8 NeuronCores — each core owns a
[1024, 2048] slice of the output and needs no cross-core communication
(softmax reduces over keys, which are fully local).

Per-core algorithm:
  - gate[k] = sigmoid(span[k]) / sqrt(D) is folded into K rows during the
    fp32->bf16 convert (per-partition scalar multiply).
  - Q/K/V are loaded with GpSimd SWDGE cast-DMAs (fp32 DRAM -> bf16
    SBUF, no compute-engine converts); Q^T / K^T are produced with PE
    transposes (batched 8 blocks per PSUM bank, copied out on ACT); P^T
    uses the DMA XBAR transpose in phase 2 where the DMA fabric is
    otherwise idle.  Keeping the XBAR off the phase-1 critical path
    avoids xbar-mode serialization against the input load stream.
  - softmax skips the max-subtraction (scores are ~N(0, 0.73); exp is
    safe in fp32) and defers normalization to a per-row reciprocal
    multiply on the PV output.
"""

import sys

import numpy as np

if "/opt/trn_rl_repo" not in sys.path:
    sys.path.insert(0, "/opt/trn_rl_repo")

B = 4
SEQ = 2048
D = 2048
N_CORES = 8
Q_SHARD = (B * SEQ) // N_CORES  # 1024 query rows per core

_CACHE: dict = {}


def build(q_rows: int = Q_SHARD, seq: int = SEQ, d: int = D):
    """Build + compile the per-core Bass graph (same graph on all cores)."""
    import ml_dtypes

    import concourse.tile as tile
    from concourse import bacc, mybir

    f32 = mybir.dt.float32
    bf16 = mybir.dt.bfloat16
    AF = mybir.ActivationFunctionType

    P = 128
    n_qt = q_rows // P
    n_kt = seq // P
    n_dt = d // P
    KC = 512  # k-chunk width (one PSUM bank of fp32 scores)
    n_kc = seq // KC
    kt_per_kc = KC // P
    DC = 512  # output d-chunk width
    n_dc = d // DC
    TB = 8  # transpose blocks batched per PSUM bank
    scale = 1.0 / float(np.sqrt(d))

    nc = bacc.Bacc("TRN2", target_bir_lowering=False, debug=False)
    q_d = nc.dram_tensor("q", [q_rows, d], f32, kind="ExternalInput").ap()
    k_d = nc.dram_tensor("k", [seq, d], f32, kind="ExternalInput").ap()
    v_d = nc.dram_tensor("v", [seq, d], f32, kind="ExternalInput").ap()
    span_d = nc.dram_tensor("span", [1, seq], f32, kind="ExternalInput").ap()
    out_d = nc.dram_tensor("out", [q_rows, d], f32, kind="ExternalOutput").ap()

    with tile.TileContext(nc) as tc:
        with tc.tile_pool(name="singles", bufs=1) as singles, \
             tc.tile_pool(name="cv", bufs=13) as cvp, \
             tc.tile_pool(name="ktp", bufs=2) as ktp, \
             tc.tile_pool(name="ptp", bufs=2) as ptp, \
             tc.tile_pool(name="obp", bufs=2) as obp, \
             tc.tile_pool(name="trps", bufs=2, space="PSUM") as trps:

            # Identity as a NEFF-embedded constant: loads on the HWDGE
            # queue so the GpSimd SWDGE (cast-DMA) queue starts immediately.
            ident_dram = nc.inline_tensor(
                np.eye(P, dtype=ml_dtypes.bfloat16), name="ident_c"
            )
            ident = singles.tile([P, P], bf16, tag="ident")
            nc.sync.dma_start(out=ident, in_=ident_dram.ap())

            # gate[p, t] = sigmoid(span[t*128 + p]) * scale.
            # Load the span row contiguously (one descriptor -- the
            # [1,2048]->[128,16] gather costs ~25us of 4-byte descriptors
            # and was the prologue critical path), sigmoid on the row,
            # then distribute to partitions with 16 tiny PE outer-product
            # matmuls against a [1,1] ones tile.
            span_row = singles.tile([1, seq], f32, tag="span_row")
            nc.sync.dma_start(out=span_row, in_=span_d)
            ones11 = singles.tile([1, 1], f32, tag="ones11")
            nc.vector.memset(ones11, 1.0)
            gate = singles.tile([P, n_kt], f32, tag="gate")

            # Persistent bf16 tensors
            QT = singles.tile([P, n_dt, q_rows], bf16, tag="QT")  # [d, dt, q]
            Vb = singles.tile([P, n_kt, d], bf16, tag="Vb")       # [k, kt, d]
            Pm = singles.tile([P, n_qt, seq], bf16, tag="Pm")     # [q, qt, k]
            sums = singles.tile([P, n_qt, n_kc], f32, tag="sums")

            # Loads are split into transpose-group-sized pieces so each
            # PE transpose group can start as soon as its piece lands.
            tb = min(TB, n_dt)
            n_tg = n_dt // tb
            GW = tb * P  # columns per transpose group

            def load_q(qt):
                # SWDGE cast-DMA: fp32 DRAM -> bf16 SBUF directly
                pieces = []
                for g in range(n_tg):
                    t = cvp.tile([P, GW], bf16, tag="cv", name=f"qr{qt}_{g}")
                    nc.gpsimd.dma_start(
                        out=t, in_=q_d[qt * P:(qt + 1) * P, g * GW:(g + 1) * GW]
                    )
                    pieces.append(t)
                return pieces

            def load_k(kt):
                pieces = []
                for g in range(n_tg):
                    t = cvp.tile([P, GW], bf16, tag="cv", name=f"kr{kt}_{g}")
                    nc.gpsimd.dma_start(
                        out=t, in_=k_d[kt * P:(kt + 1) * P, g * GW:(g + 1) * GW]
                    )
                    # per-key gate fold in place (bf16 4x DVE mode)
                    nc.vector.tensor_scalar_mul(t, t, gate[:, kt:kt + 1])
                    pieces.append(t)
                return pieces

            def load_v(kt):
                nc.gpsimd.dma_start(out=Vb[:, kt, :], in_=v_d[kt * P:(kt + 1) * P, :])

            def pe_transpose(pieces, dst, dst_col, copy_engine):
                """dst[:, dt, dst_col*128 : +128] = pieces^T blocks.

                pieces: n_tg tiles of [128, tb*128] bf16 (natural layout).
                Batches tb 128x128 PE transposes per PSUM bank, then one
                copy per bank to SBUF.
                """
                for g in range(n_tg):
                    tr = trps.tile([P, tb, P], bf16, tag="tr")
                    for j in range(tb):
                        nc.tensor.transpose(
                            tr[:, j, :], pieces[g][:, j * P:(j + 1) * P], ident
                        )
                    copy_engine(
                        out=dst[:, g * tb:(g + 1) * tb,
                                dst_col * P:(dst_col + 1) * P],
                        in_=tr,
                    )

            def s_block(kc, qt, KT):
                s_ps = spsum.tile([P, KC], f32, tag="s")
                for dt in range(n_dt):
                    nc.tensor.matmul(
                        s_ps,
                        QT[:, dt, qt * P:(qt + 1) * P],
                        KT[:, dt, :],
                        start=(dt == 0),
                        stop=(dt == n_dt - 1),
                    )
                nc.scalar.activation(
                    out=Pm[:, qt, kc * KC:(kc + 1) * KC],
                    in_=s_ps,
                    func=AF.Exp,
                    accum_out=sums[:, qt, kc:kc + 1],
                )

            gp = tc.tile_pool(name="gatep", bufs=1, space="PSUM")
            gpp = gp.__enter__()
            gate_ps = gpp.tile([P, n_kt], f32, tag="gps")
            for t in range(n_kt):
                nc.tensor.matmul(
                    gate_ps[:, t:t + 1],
                    span_row[0:1, t * P:(t + 1) * P],
                    ones11,
                    start=True,
                    stop=True,
                )
            # sigmoid AFTER distribution: runs on 128 lanes instead of 1,
            # and the PE matmuls start as soon as the span row lands
            nc.scalar.activation(out=gate, in_=gate_ps, func=AF.Sigmoid)
            nc.vector.tensor_scalar_mul(gate, gate, scale)
            gp.__exit__(None, None, None)

            ph1 = tc.tile_pool(name="spsum", bufs=5, space="PSUM")
            spsum = ph1.__enter__()
            # ---- Phase 1: S = gated Q K^T, P = exp(S) ------------------
            # SWDGE (cast-DMA) queue is FIFO: emit loads in consumption
            # order (K chunk 0 first -- its 64 PE transposes are the long
            # pole to the first S matmul), prefetch K one chunk ahead, and
            # push all V loads to the tail (phase 2 consumes V much later).
            # The next chunk's transposes are interleaved into the current
            # chunk's S loop so KT production never stalls the PE.
            kbs = {0: [load_k(j) for j in range(kt_per_kc)]}
            qb0 = load_q(0)
            qb1 = load_q(1)
            if n_kc > 1:
                kbs[1] = [load_k(kt_per_kc + j) for j in range(kt_per_kc)]

            KTs = {}
            ktcopy = nc.scalar.copy

            def mk_tk(kc, j):
                if j == 0:
                    KTs[kc] = ktp.tile([P, n_dt, KC], bf16, tag="KT", name=f"KT{kc}")
                pe_transpose(kbs[kc][j], KTs[kc], j, ktcopy)

            for j in range(kt_per_kc):
                mk_tk(0, j)
            pe_transpose(qb0, QT, 0, nc.scalar.copy)
            pe_transpose(qb1, QT, 1, nc.scalar.copy)
            if n_qt > 2:  # transpose Q several tiles ahead of its S block
                pe_transpose(load_q(2), QT, 2, nc.scalar.copy)
            for qt in range(n_qt):
                s_block(0, qt, KTs[0])
                if qt + 3 < n_qt:
                    pe_transpose(load_q(qt + 3), QT, qt + 3, nc.scalar.copy)
                if qt - (n_qt - kt_per_kc) >= 0 and n_kc > 1:
                    mk_tk(1, qt - (n_qt - kt_per_kc))
                if qt == n_qt - 1 and n_kc > 2:
                    kbs[2] = [
                        load_k(2 * kt_per_kc + j) for j in range(kt_per_kc)
                    ]

            for kc in range(1, n_kc):
                for qt in range(n_qt):
                    s_block(kc, qt, KTs[kc])
                    if qt < kt_per_kc and kc + 1 < n_kc:
                        mk_tk(kc + 1, qt)
                    if qt == 0 and kc + 2 < n_kc:
                        kbs[kc + 2] = [
                            load_k((kc + 2) * kt_per_kc + j)
                            for j in range(kt_per_kc)
                        ]

            for vt in range(n_kt):
                load_v(vt)

            ph1.__exit__(None, None, None)
            ph2 = tc.tile_pool(name="opsum", bufs=6, space="PSUM")
            opsum = ph2.__enter__()

            rowsum = singles.tile([P, n_qt], f32, tag="rowsum")
            nc.vector.tensor_reduce(
                out=rowsum, in_=sums, axis=mybir.AxisListType.X,
                op=mybir.AluOpType.add,
            )
            rinv = singles.tile([P, n_qt], f32, tag="rinv")
            nc.vector.reciprocal(rinv, rowsum)

            # ---- Phase 2: O[qt] = (P[qt] @ V) * rinv[qt] ---------------
            for qt in range(n_qt):
                PT = ptp.tile([P, n_kt, P], bf16, tag="PT")
                nc.scalar.dma_start_transpose(out=PT, in_=Pm[:, qt, :])
                for dc in range(n_dc):
                    o_ps = opsum.tile([P, DC], f32, tag="o")
                    for kt in range(n_kt):
                        nc.tensor.matmul(
                            o_ps,
                            PT[:, kt, :],
                            Vb[:, kt, dc * DC:(dc + 1) * DC],
                            start=(kt == 0),
                            stop=(kt == n_kt - 1),
                        )
                    ob = obp.tile([P, DC], f32, tag="ob")
                    nc.vector.tensor_scalar_mul(ob, o_ps, rinv[:, qt:qt + 1])
                    nc.sync.dma_start(
                        out=out_d[qt * P:(qt + 1) * P, dc * DC:(dc + 1) * DC],
                        in_=ob,
                    )

            ph2.__exit__(None, None, None)

    nc.compile()
    return nc


def _get_compiled():
    if "nc" not in _CACHE:
        _CACHE["nc"] = build()
    return _CACHE["nc"]


def _shard_inputs(query, key, value, span):
    in_maps = []
    for c in range(N_CORES):
        b, h = c // 2, c % 2
        in_maps.append({
            "q": np.ascontiguousarray(
                query[b, h * Q_SHARD:(h + 1) * Q_SHARD], dtype=np.float32
            ),
            "k": np.ascontiguousarray(key[b], dtype=np.float32),
            "v": np.ascontiguousarray(value[b], dtype=np.float32),
            "span": np.ascontiguousarray(span, dtype=np.float32),
        })
    return in_maps


def kernel(**inputs) -> np.ndarray:
    query = np.asarray(inputs["query"], dtype=np.float32)
    key = np.asarray(inputs["key"], dtype=np.float32)
    value = np.asarray(inputs["value"], dtype=np.float32)
    span = np.asarray(inputs["span_param"], dtype=np.float32)

    from concourse.bass_utils import run_bass_kernel_spmd

    nc = _get_compiled()
    in_maps = _shard_inputs(query, key, value, span)
    res = run_bass_kernel_spmd(nc, in_maps, core_ids=list(range(N_CORES)))

    out = np.empty((B, SEQ, D), dtype=np.float32)
    for c in range(N_CORES):
        b, h = c // 2, c % 2
        out[b, h * Q_SHARD:(h + 1) * Q_SHARD] = res.results[c]["out"]
    return out


if __name__ == "__main__":
    rng = np.random.default_rng(0)
    inputs = {
        "query": rng.standard_normal((B, SEQ, D), dtype=np.float32),
        "key": rng.standard_normal((B, SEQ, D), dtype=np.float32),
        "value": rng.standard_normal((B, SEQ, D), dtype=np.float32),
        "span_param": np.ones((1, SEQ), dtype=np.float32),
    }
    out = kernel(**inputs)
    print(out.shape, out.dtype, float(np.abs(out).mean()))
